# revision 6
# baseline (speedup 1.0000x reference)
"""MultiHeadAttentionPool3D on 8 Trainium2 NeuronCores.

Math (per batch b):
  scores[hq, s] = scale * (q_eff[hq, :] @ x[b, :, s])     (key-projection folded into
                                                           the queries; per-row bias
                                                           terms cancel in softmax)
  p = exp(scores)       (no max-subtraction: scores ~ N(0,1), fp32-safe)
  l[hq] = sum_s p[hq, s];   y[hq, c] = sum_s p[hq, s] * x[b, c, s]
  pooled = y / l  -> tiny epilogue (value proj, Wo, layernorm) on host.

Sharding: core = b * 2 + s_half  (4 batches x 2 halves of S=32768).

v9 design (both matmuls in small-output orientation):
  - host passes TWO fp16 layouts of the shard: x [C, S_loc] (c on partitions)
    and a flat transposed plane xt [128, n_sb*256] (s on partitions; column
    block j = x[:, j*128:(j+1)*128].T).
  - scoresT per s-block DIRECTLY via x-tile-as-stationary:
      psum_sT[:, sb*32:+32] += x_half[:, sb*128:+128].T @ q_effT_half
    16 s-blocks packed per [128, 512] PSUM bank; the output IS pT-oriented,
    so p never needs transposing.
  - p = Exp(scale*scoresT): one [128,512] ScalarE activation per chunk -> fp16.
  - l partials: ones[128,1].T @ pT_chunk -> psum_l [1, 512], accumulated
    across chunks (host sums the 16 slots).
  - y: per s-block and c-half: xt_slice[s,128].T @ pT_slice[s,32]
    -> psum_y [128(c-half), 64(2*hq)], PSUM-accumulated over all s.
  - host reassembles y/l and runs the tiny epilogue.
"""

import sys

if "/opt/trn_rl_repo" not in sys.path:
    sys.path.insert(0, "/opt/trn_rl_repo")

import numpy as np

NUM_HEADS = 8
OUT_FEATURES = 512
NUM_QUERIES = 4
C = 256
HEAD_DIM = OUT_FEATURES // NUM_HEADS
LN_EPS = 1e-5
B = 4
S = 32 * 32 * 32
N_CORES = 8
S_LOC = S // 2  # shard: (batch, half of spatial axis)
HQ = NUM_HEADS * NUM_QUERIES  # 32 fused query rows, hq = h*NUM_QUERIES + q
SCALE = HEAD_DIM ** -0.5
CHUNK = 2048

_NC_CACHE = {}


def _build_nc(s_loc=S_LOC, chunk=CHUNK, loop_n=1, x_f32=False, multi_queue=True, dbg_swap_h=False, dbg_no_l=False):
    import concourse.bass as bass
    import concourse.tile as tile
    from concourse import bacc, mybir
    import contextlib

    f32 = mybir.dt.float32
    f16 = mybir.dt.float16
    # x planes travel as fp8 e3m4 (halves HBM traffic); q_eff and p stay
    # fp16 — mixed-dtype matmuls (fp8 stationary x fp16 moving) are exact
    # on HW and q/p quantization is the accuracy-critical path.
    xdt = f32 if x_f32 else mybir.dt.float8e3
    Exp = mybir.ActivationFunctionType.Exp

    if isinstance(chunk, int):
        assert s_loc % chunk == 0 and chunk % 512 == 0
        sizes = [chunk] * (s_loc // chunk)
    else:
        sizes = list(chunk)
        assert sum(sizes) == s_loc and all(c % 512 == 0 for c in sizes)
    assert sizes[0] >= 2048, "first chunk must init the full l zero-region"
    n_ch = len(sizes)
    n_sb = s_loc // 128
    W = 256

    qdt = f32 if x_f32 else f16
    nc = bacc.Bacc("TRN2", target_bir_lowering=False, debug=False,
                   num_devices=N_CORES)
    x_d = nc.dram_tensor("x", [C, s_loc], xdt, kind="ExternalInput")
    xt_d = nc.dram_tensor("xt", [128, n_sb * W], xdt, kind="ExternalInput")
    qT_d = nc.dram_tensor("qT", [C, HQ], qdt, kind="ExternalInput")
    y_d = nc.dram_tensor("y", [128, 2 * HQ], f32, kind="ExternalOutput")
    l_d = nc.dram_tensor("l", [1, 512], f32, kind="ExternalOutput")

    with tile.TileContext(nc) as tc:
        with (
            tc.tile_pool(name="const", bufs=1) as constp,
            tc.tile_pool(name="xstage", bufs=3) as xstage,
            tc.tile_pool(name="xtstage", bufs=3) as xtstage,
            tc.tile_pool(name="ptstage", bufs=3) as ptstage,
            tc.tile_pool(name="outp", bufs=2) as outp,
            tc.tile_pool(name="ps_st", bufs=3, space="PSUM") as ps_st,
            tc.tile_pool(name="ps_y", bufs=1, space="PSUM") as ps_yp,
            tc.tile_pool(name="ps_l", bufs=1, space="PSUM") as ps_lp,
        ):
            qt0 = constp.tile([128, HQ], qdt)
            nc.sync.dma_start(qt0[:], qT_d[0:128, :])
            qt1 = constp.tile([128, HQ], qdt)
            nc.sync.dma_start(qt1[:], qT_d[128:256, :])
            ones = constp.tile([128, 1], f16)
            nc.gpsimd.memset(ones[:], 1.0)

            def iter_scope():
                if loop_n > 1:
                    E = mybir.EngineType
                    return tc.For_i(0, loop_n, 1,
                                    hint_engines=(E.PE, E.DVE, E.Activation,
                                                  E.SP, E.Pool))
                return contextlib.nullcontext()

            q0 = nc.sync
            q1 = nc.scalar if multi_queue else nc.sync
            with iter_scope():
                psum_y = ps_yp.tile([128, 2 * HQ], f32, tag="psy")
                psum_l = ps_lp.tile([1, 512], f32, tag="psl")

                offs = [sum(sizes[:i]) for i in range(n_ch)]
                for k in range(n_ch):
                    o = offs[k]
                    ck = sizes[k]
                    sbpc = ck // 128
                    xc0 = xstage.tile([128, ck], xdt, tag="xc0")
                    q0.dma_start(xc0[:], x_d[0:128, o:o + ck])
                    xc1 = xstage.tile([128, ck], xdt, tag="xc1")
                    q0.dma_start(xc1[:], x_d[128:256, o:o + ck])
                    xt_c = xtstage.tile([128, sbpc * W], xdt, tag="xt")
                    ocol = (o // 128) * W
                    q1.dma_start(xt_c[:], xt_d[:, ocol:ocol + sbpc * W])

                    # scoresT: 16 s-blocks packed into one [128, 512] bank
                    pst = ps_st.tile([128, sbpc * HQ], f32, tag="pst")
                    for sb in range(sbpc):
                        for h, (xc, qt) in enumerate(((xc0, qt0), (xc1, qt1))):
                            nc.tensor.matmul(
                                pst[:, sb * HQ:(sb + 1) * HQ],
                                xc[:, sb * 128:(sb + 1) * 128], qt[:],
                                start=(sb == 0 and h == 0),
                                stop=(sb == sbpc - 1 and h == 1),
                                skip_group_check=True)
                    pt_c = ptstage.tile([128, sbpc * HQ], f16, tag="pt")
                    nc.scalar.activation(pt_c[:], pst[:], Exp, scale=SCALE)
                    # l partials: sum over the 128 s-rows of this chunk
                    if not dbg_no_l:
                        ngr = (sbpc + 15) // 16
                        for g in range(ngr):
                            wsb = min(16, sbpc - g * 16)
                            nc.tensor.matmul(
                                psum_l[:, 0:wsb * HQ], ones[:],
                                pt_c[:, g * 16 * HQ:(g * 16 + wsb) * HQ],
                                start=(k == 0 and g == 0),
                                stop=(k == n_ch - 1 and g == ngr - 1),
                                skip_group_check=True)
                    else:
                        nc.gpsimd.memset(psum_l[:], 0.0) if k == 0 else None
                    # y: [c-half, hq] accumulated over all s-blocks
                    for sb in range(sbpc):
                        gsb = o // 128 + sb
                        for h in ((1, 0) if dbg_swap_h else (0, 1)):
                            nc.tensor.matmul(
                                psum_y[:, h * HQ:(h + 1) * HQ],
                                xt_c[:, sb * W + h * 128:sb * W + (h + 1) * 128],
                                pt_c[:, sb * HQ:(sb + 1) * HQ],
                                start=(gsb == 0 and h == (1 if dbg_swap_h else 0)),
                                stop=(gsb == n_sb - 1 and h == (0 if dbg_swap_h else 1)),
                                skip_group_check=True)

                y_t = outp.tile([128, 2 * HQ], f32, tag="yt")
                nc.vector.tensor_copy(y_t[:], psum_y[:])
                l_t = outp.tile([1, 512], f32, tag="lt")
                nc.vector.tensor_copy(l_t[:], psum_l[:])
                nc.sync.dma_start(y_d[:], y_t[:])
                nc.sync.dma_start(l_d[:], l_t[:])

    nc.compile()
    return nc


def _get_nc(loop_n=1, x_f32=False, chunk=CHUNK, multi_queue=True):
    key = (S_LOC, loop_n, x_f32, chunk, multi_queue)
    if key not in _NC_CACHE:
        _NC_CACHE[key] = _build_nc(loop_n=loop_n, x_f32=x_f32, chunk=chunk,
                                   multi_queue=multi_queue)
    return _NC_CACHE[key]


def _shard_inputs(shard, qT, s_loc=S_LOC, chunk=CHUNK, x_f32=False):
    """shard: [C, s_loc] fp32 -> in_map for one core."""
    import ml_dtypes

    n_sb = s_loc // 128
    xdt = np.float32 if x_f32 else ml_dtypes.float8_e3m4
    x8 = shard.astype(xdt)
    # flat transposed plane: column block j (width C) = shard[:, j*128:+128].T
    xt = np.ascontiguousarray(
        shard.T.reshape(n_sb, 128, C).astype(xdt)
        .transpose(1, 0, 2).reshape(128, n_sb * C))
    return {"x": np.ascontiguousarray(x8), "xt": xt,
            "qT": qT.astype(np.float32 if x_f32 else np.float16)}


def _prepare_in_maps(x, queries, Wk, x_f32=False):
    xf = np.ascontiguousarray(np.asarray(x, np.float32).reshape(B, C, S))
    qr = np.asarray(queries, np.float32).reshape(NUM_QUERIES, NUM_HEADS, HEAD_DIM)
    Wkr = np.asarray(Wk, np.float32).reshape(NUM_HEADS, HEAD_DIM, C)
    # q_eff[h*NQ+q, c] = sum_d q[q,h,d] * Wk[h*hd+d, c]
    q_eff = np.einsum("qhd,hdc->hqc", qr, Wkr).reshape(HQ, C)
    qT = np.ascontiguousarray(q_eff.T.astype(np.float32))
    in_maps = []
    for core in range(N_CORES):
        b, half = divmod(core, 2)
        shard = np.ascontiguousarray(xf[b, :, half * S_LOC:(half + 1) * S_LOC])
        in_maps.append(_shard_inputs(shard, qT, x_f32=x_f32))
    return in_maps


def _extract_yl(yv, lv):
    """Device outputs -> (Y [HQ, C], L [HQ]) for one core."""
    Y = np.concatenate([yv[:, 0:HQ].T, yv[:, HQ:2 * HQ].T], axis=1)  # [HQ, 256]
    L = lv.reshape(-1, HQ).sum(axis=0)
    return Y, L


def _epilogue(Y, L, Wv, bv, Wo, bo, gamma, beta):
    """Y [B, HQ, C], L [B, HQ] -> final [B, OUT_FEATURES]."""
    pooled = (Y / L[:, :, None]).reshape(B, NUM_HEADS, NUM_QUERIES, C)
    Wvr = np.asarray(Wv, np.float32).reshape(NUM_HEADS, HEAD_DIM, C)
    att = np.einsum("hdc,bhqc->bhqd", Wvr, pooled)
    att += np.asarray(bv, np.float32).reshape(1, NUM_HEADS, 1, HEAD_DIM)
    multi = att.transpose(0, 2, 1, 3).reshape(B, NUM_QUERIES * OUT_FEATURES)
    out = multi @ np.asarray(Wo, np.float32).T + np.asarray(bo, np.float32)
    mu = out.mean(-1, keepdims=True)
    var = ((out - mu) ** 2).mean(-1, keepdims=True)
    out = (out - mu) / np.sqrt(var + LN_EPS)
    out = out * np.asarray(gamma, np.float32) + np.asarray(beta, np.float32)
    return out.astype(np.float32)


def kernel(x, queries, Wk, bk, Wv, bv, Wo, bo, gamma, beta):
    from concourse.bass_utils import run_bass_kernel_spmd

    in_maps = _prepare_in_maps(x, queries, Wk)
    nc = _get_nc()
    res = run_bass_kernel_spmd(nc, in_maps, list(range(N_CORES))).results
    Y = np.zeros((B, HQ, C), np.float32)
    L = np.zeros((B, HQ), np.float32)
    for core in range(N_CORES):
        b = core // 2
        Yc, Lc = _extract_yl(res[core]["y"], res[core]["l"])
        Y[b] += Yc
        L[b] += Lc
    return _epilogue(Y, L, Wv, bv, Wo, bo, gamma, beta)



# revision 7
# speedup vs baseline: 1.0054x; 1.0054x over previous
"""MultiHeadAttentionPool3D on 8 Trainium2 NeuronCores.

Math (per batch b):
  scores[hq, s] = scale * (q_eff[hq, :] @ x[b, :, s])     (key-projection folded into
                                                           the queries; per-row bias
                                                           terms cancel in softmax)
  p = exp(scores)       (no max-subtraction: scores ~ N(0,1), fp32-safe)
  l[hq] = sum_s p[hq, s];   y[hq, c] = sum_s p[hq, s] * x[b, c, s]
  pooled = y / l  -> tiny epilogue (value proj, Wo, layernorm) on host.

Sharding: core = b * 2 + s_half  (4 batches x 2 halves of S=32768).

v9 design (both matmuls in small-output orientation):
  - host passes TWO fp16 layouts of the shard: x [C, S_loc] (c on partitions)
    and a flat transposed plane xt [128, n_sb*256] (s on partitions; column
    block j = x[:, j*128:(j+1)*128].T).
  - scoresT per s-block DIRECTLY via x-tile-as-stationary:
      psum_sT[:, sb*32:+32] += x_half[:, sb*128:+128].T @ q_effT_half
    16 s-blocks packed per [128, 512] PSUM bank; the output IS pT-oriented,
    so p never needs transposing.
  - p = Exp(scale*scoresT): one [128,512] ScalarE activation per chunk -> fp16.
  - l partials: ones[128,1].T @ pT_chunk -> psum_l [1, 512], accumulated
    across chunks (host sums the 16 slots).
  - y: per s-block and c-half: xt_slice[s,128].T @ pT_slice[s,32]
    -> psum_y [128(c-half), 64(2*hq)], PSUM-accumulated over all s.
  - host reassembles y/l and runs the tiny epilogue.
"""

import sys

if "/opt/trn_rl_repo" not in sys.path:
    sys.path.insert(0, "/opt/trn_rl_repo")

import numpy as np

NUM_HEADS = 8
OUT_FEATURES = 512
NUM_QUERIES = 4
C = 256
HEAD_DIM = OUT_FEATURES // NUM_HEADS
LN_EPS = 1e-5
B = 4
S = 32 * 32 * 32
N_CORES = 8
S_LOC = S // 2  # shard: (batch, half of spatial axis)
HQ = NUM_HEADS * NUM_QUERIES  # 32 fused query rows, hq = h*NUM_QUERIES + q
SCALE = HEAD_DIM ** -0.5
CHUNK = 2048

_NC_CACHE = {}


def _build_nc(s_loc=S_LOC, chunk=CHUNK, dma_chunk=4096, loop_n=1, x_f32=False,
              multi_queue=True, pipe=True):
    import concourse.bass as bass
    import concourse.tile as tile
    from concourse import bacc, mybir
    import contextlib

    f32 = mybir.dt.float32
    f16 = mybir.dt.float16
    # x planes travel as fp8 e3m4 (halves HBM traffic); q_eff and p stay
    # fp16 — mixed-dtype matmuls (fp8 stationary x fp16 moving) are exact
    # on HW and q/p quantization is the accuracy-critical path.
    xdt = f32 if x_f32 else mybir.dt.float8e3
    Exp = mybir.ActivationFunctionType.Exp

    assert s_loc % dma_chunk == 0 and dma_chunk % chunk == 0
    assert chunk == 2048, "pst/l packing assumes 16 s-blocks per sub-chunk"
    n_ch = s_loc // chunk
    per_dma = dma_chunk // chunk
    n_sb = s_loc // 128
    W = 256
    sbpc = chunk // 128

    qdt = f32 if x_f32 else f16
    nc = bacc.Bacc("TRN2", target_bir_lowering=False, debug=False,
                   num_devices=N_CORES)
    x_d = nc.dram_tensor("x", [C, s_loc], xdt, kind="ExternalInput")
    xt_d = nc.dram_tensor("xt", [128, n_sb * W], xdt, kind="ExternalInput")
    qT_d = nc.dram_tensor("qT", [C, HQ], qdt, kind="ExternalInput")
    y_d = nc.dram_tensor("y", [128, 2 * HQ], f32, kind="ExternalOutput")
    l_d = nc.dram_tensor("l", [1, 512], f32, kind="ExternalOutput")

    with tile.TileContext(nc) as tc:
        with (
            tc.tile_pool(name="const", bufs=1) as constp,
            tc.tile_pool(name="xstage", bufs=3) as xstage,
            tc.tile_pool(name="xtstage", bufs=3) as xtstage,
            tc.tile_pool(name="ptstage", bufs=3) as ptstage,
            tc.tile_pool(name="outp", bufs=2) as outp,
            tc.tile_pool(name="ps_st", bufs=3, space="PSUM") as ps_st,
            tc.tile_pool(name="ps_y", bufs=2, space="PSUM") as ps_yp,
            tc.tile_pool(name="ps_l", bufs=2, space="PSUM") as ps_lp,
        ):
            qt0 = constp.tile([128, HQ], qdt)
            nc.sync.dma_start(qt0[:], qT_d[0:128, :])
            qt1 = constp.tile([128, HQ], qdt)
            nc.sync.dma_start(qt1[:], qT_d[128:256, :])
            ones = constp.tile([128, 1], f16)
            nc.gpsimd.memset(ones[:], 1.0)

            def iter_scope():
                if loop_n > 1:
                    E = mybir.EngineType
                    return tc.For_i(0, loop_n, 1,
                                    hint_engines=(E.PE, E.DVE, E.Activation,
                                                  E.SP, E.Pool))
                return contextlib.nullcontext()

            q0 = nc.sync
            q1 = nc.scalar if multi_queue else nc.sync
            with iter_scope():
                psum_y = ps_yp.tile([128, 2 * HQ], f32, tag="psy")
                psum_l = ps_lp.tile([1, 512], f32, tag="psl")

                def emit_l(kk, pt_c):
                    nc.tensor.matmul(
                        psum_l[:], ones[:], pt_c[:],
                        start=(kk == 0), stop=(kk == n_ch - 1),
                        skip_group_check=True)

                def emit_mm2(kk, pt_c, xt_t, xt_lo):
                    for sb in range(sbpc):
                        gsb = kk * sbpc + sb
                        for h in (0, 1):
                            nc.tensor.matmul(
                                psum_y[:, h * HQ:(h + 1) * HQ],
                                xt_t[:, xt_lo + sb * W + h * 128:
                                     xt_lo + sb * W + (h + 1) * 128],
                                pt_c[:, sb * HQ:(sb + 1) * HQ],
                                start=(gsb == 0 and h == 0),
                                stop=(gsb == n_sb - 1 and h == 1),
                                skip_group_check=True)

                pending = None  # (k, pt_c, xt_t, xt_lo) awaiting l+mm2
                xc0_t = xc1_t = xt_t = None
                for k in range(n_ch):
                    if k % per_dma == 0:
                        d_o = k * chunk
                        xc0_t = xstage.tile([128, dma_chunk], xdt, tag="xc0")
                        q0.dma_start(xc0_t[:], x_d[0:128, d_o:d_o + dma_chunk])
                        xc1_t = xstage.tile([128, dma_chunk], xdt, tag="xc1")
                        q0.dma_start(xc1_t[:], x_d[128:256, d_o:d_o + dma_chunk])
                        dcols = (dma_chunk // 128) * W
                        xt_t = xtstage.tile([128, dcols], xdt, tag="xt")
                        ocol = (d_o // 128) * W
                        q1.dma_start(xt_t[:], xt_d[:, ocol:ocol + dcols])
                    lo = (k % per_dma) * chunk
                    xt_lo = (k % per_dma) * sbpc * W

                    # scoresT: 16 s-blocks packed into one [128, 512] bank
                    pst = ps_st.tile([128, sbpc * HQ], f32, tag="pst")
                    for sb in range(sbpc):
                        for h, (xc, qt) in enumerate(((xc0_t, qt0), (xc1_t, qt1))):
                            nc.tensor.matmul(
                                pst[:, sb * HQ:(sb + 1) * HQ],
                                xc[:, lo + sb * 128:lo + (sb + 1) * 128], qt[:],
                                start=(sb == 0 and h == 0),
                                stop=(sb == sbpc - 1 and h == 1),
                                skip_group_check=True)
                    pt_c = ptstage.tile([128, sbpc * HQ], f16, tag="pt")
                    nc.scalar.activation(pt_c[:], pst[:], Exp, scale=SCALE)
                    if pipe:
                        # l+mm2 run one sub-chunk behind so exp(k) overlaps
                        # mm1(k+1) instead of stalling PE.
                        if pending is not None:
                            emit_l(*pending[:2])
                            emit_mm2(*pending)
                        pending = (k, pt_c, xt_t, xt_lo)
                    else:
                        emit_l(k, pt_c)
                        emit_mm2(k, pt_c, xt_t, xt_lo)
                if pending is not None:
                    emit_l(*pending[:2])
                    emit_mm2(*pending)

                y_t = outp.tile([128, 2 * HQ], f32, tag="yt")
                nc.vector.tensor_copy(y_t[:], psum_y[:])
                l_t = outp.tile([1, 512], f32, tag="lt")
                nc.vector.tensor_copy(l_t[:], psum_l[:])
                nc.sync.dma_start(y_d[:], y_t[:])
                nc.sync.dma_start(l_d[:], l_t[:])

    nc.compile()
    return nc


def _get_nc(loop_n=1, x_f32=False, chunk=CHUNK, dma_chunk=4096,
            multi_queue=True, pipe=True):
    key = (S_LOC, loop_n, x_f32, chunk, dma_chunk, multi_queue, pipe)
    if key not in _NC_CACHE:
        _NC_CACHE[key] = _build_nc(loop_n=loop_n, x_f32=x_f32, chunk=chunk,
                                   dma_chunk=dma_chunk,
                                   multi_queue=multi_queue, pipe=pipe)
    return _NC_CACHE[key]


def _shard_inputs(shard, qT, s_loc=S_LOC, chunk=CHUNK, x_f32=False):
    """shard: [C, s_loc] fp32 -> in_map for one core."""
    import ml_dtypes

    n_sb = s_loc // 128
    xdt = np.float32 if x_f32 else ml_dtypes.float8_e3m4
    x8 = shard.astype(xdt)
    # flat transposed plane: column block j (width C) = shard[:, j*128:+128].T
    xt = np.ascontiguousarray(
        shard.T.reshape(n_sb, 128, C).astype(xdt)
        .transpose(1, 0, 2).reshape(128, n_sb * C))
    return {"x": np.ascontiguousarray(x8), "xt": xt,
            "qT": qT.astype(np.float32 if x_f32 else np.float16)}


def _prepare_in_maps(x, queries, Wk, x_f32=False):
    xf = np.ascontiguousarray(np.asarray(x, np.float32).reshape(B, C, S))
    qr = np.asarray(queries, np.float32).reshape(NUM_QUERIES, NUM_HEADS, HEAD_DIM)
    Wkr = np.asarray(Wk, np.float32).reshape(NUM_HEADS, HEAD_DIM, C)
    # q_eff[h*NQ+q, c] = sum_d q[q,h,d] * Wk[h*hd+d, c]
    q_eff = np.einsum("qhd,hdc->hqc", qr, Wkr).reshape(HQ, C)
    qT = np.ascontiguousarray(q_eff.T.astype(np.float32))
    in_maps = []
    for core in range(N_CORES):
        b, half = divmod(core, 2)
        shard = np.ascontiguousarray(xf[b, :, half * S_LOC:(half + 1) * S_LOC])
        in_maps.append(_shard_inputs(shard, qT, x_f32=x_f32))
    return in_maps


def _extract_yl(yv, lv):
    """Device outputs -> (Y [HQ, C], L [HQ]) for one core."""
    Y = np.concatenate([yv[:, 0:HQ].T, yv[:, HQ:2 * HQ].T], axis=1)  # [HQ, 256]
    L = lv.reshape(-1, HQ).sum(axis=0)
    return Y, L


def _epilogue(Y, L, Wv, bv, Wo, bo, gamma, beta):
    """Y [B, HQ, C], L [B, HQ] -> final [B, OUT_FEATURES]."""
    pooled = (Y / L[:, :, None]).reshape(B, NUM_HEADS, NUM_QUERIES, C)
    Wvr = np.asarray(Wv, np.float32).reshape(NUM_HEADS, HEAD_DIM, C)
    att = np.einsum("hdc,bhqc->bhqd", Wvr, pooled)
    att += np.asarray(bv, np.float32).reshape(1, NUM_HEADS, 1, HEAD_DIM)
    multi = att.transpose(0, 2, 1, 3).reshape(B, NUM_QUERIES * OUT_FEATURES)
    out = multi @ np.asarray(Wo, np.float32).T + np.asarray(bo, np.float32)
    mu = out.mean(-1, keepdims=True)
    var = ((out - mu) ** 2).mean(-1, keepdims=True)
    out = (out - mu) / np.sqrt(var + LN_EPS)
    out = out * np.asarray(gamma, np.float32) + np.asarray(beta, np.float32)
    return out.astype(np.float32)


def kernel(x, queries, Wk, bk, Wv, bv, Wo, bo, gamma, beta):
    from concourse.bass_utils import run_bass_kernel_spmd

    in_maps = _prepare_in_maps(x, queries, Wk)
    nc = _get_nc()
    res = run_bass_kernel_spmd(nc, in_maps, list(range(N_CORES))).results
    Y = np.zeros((B, HQ, C), np.float32)
    L = np.zeros((B, HQ), np.float32)
    for core in range(N_CORES):
        b = core // 2
        Yc, Lc = _extract_yl(res[core]["y"], res[core]["l"])
        Y[b] += Yc
        L[b] += Lc
    return _epilogue(Y, L, Wv, bv, Wo, bo, gamma, beta)



# revision 11
# speedup vs baseline: 1.1820x; 1.1756x over previous
"""MultiHeadAttentionPool3D on 8 Trainium2 NeuronCores.

Math (per batch b):
  scores[hq, s] = scale * (q_eff[hq, :] @ x[b, :, s])     (key-projection folded into
                                                           the queries; per-row bias
                                                           terms cancel in softmax)
  p = exp(scores)       (no max-subtraction: scores ~ N(0,1), fp32-safe)
  l[hq] = sum_s p[hq, s];   y[hq, c] = sum_s p[hq, s] * x[b, c, s]
  pooled = y / l  -> tiny epilogue (value proj, Wo, layernorm) on host.

Sharding: core = b * 2 + s_half  (4 batches x 2 halves of S=32768).

v9 design (both matmuls in small-output orientation):
  - host passes TWO fp16 layouts of the shard: x [C, S_loc] (c on partitions)
    and a flat transposed plane xt [128, n_sb*256] (s on partitions; column
    block j = x[:, j*128:(j+1)*128].T).
  - scoresT per s-block DIRECTLY via x-tile-as-stationary:
      psum_sT[:, sb*32:+32] += x_half[:, sb*128:+128].T @ q_effT_half
    16 s-blocks packed per [128, 512] PSUM bank; the output IS pT-oriented,
    so p never needs transposing.
  - p = Exp(scale*scoresT): one [128,512] ScalarE activation per chunk -> fp16.
  - l partials: ones[128,1].T @ pT_chunk -> psum_l [1, 512], accumulated
    across chunks (host sums the 16 slots).
  - y: per s-block and c-half: xt_slice[s,128].T @ pT_slice[s,32]
    -> psum_y [128(c-half), 64(2*hq)], PSUM-accumulated over all s.
  - host reassembles y/l and runs the tiny epilogue.
"""

import sys

if "/opt/trn_rl_repo" not in sys.path:
    sys.path.insert(0, "/opt/trn_rl_repo")

import numpy as np

NUM_HEADS = 8
OUT_FEATURES = 512
NUM_QUERIES = 4
C = 256
HEAD_DIM = OUT_FEATURES // NUM_HEADS
LN_EPS = 1e-5
B = 4
S = 32 * 32 * 32
N_CORES = 8
S_LOC = S // 2  # shard: (batch, half of spatial axis)
HQ = NUM_HEADS * NUM_QUERIES  # 32 fused query rows, hq = h*NUM_QUERIES + q
SCALE = HEAD_DIM ** -0.5
CHUNK = 2048

_NC_CACHE = {}


def _build_nc(s_loc=S_LOC, chunk=CHUNK, dma_chunk=4096, loop_n=1, x_f32=False,
              multi_queue=True, pipe=True, unroll=4):
    import concourse.bass as bass
    import concourse.tile as tile
    from concourse import bacc, mybir
    import contextlib

    f32 = mybir.dt.float32
    f16 = mybir.dt.float16
    # x planes travel as fp8 e3m4 (halves HBM traffic); q_eff and p stay
    # fp16 — mixed-dtype matmuls (fp8 stationary x fp16 moving) are exact
    # on HW and q/p quantization is the accuracy-critical path.
    xdt = f32 if x_f32 else mybir.dt.float8e3
    Exp = mybir.ActivationFunctionType.Exp

    assert s_loc % dma_chunk == 0 and dma_chunk % chunk == 0
    assert chunk == 2048, "pst/l packing assumes 16 s-blocks per sub-chunk"
    n_ch = s_loc // chunk
    per_dma = dma_chunk // chunk
    n_sb = s_loc // 128
    W = 256
    sbpc = chunk // 128

    qdt = f32 if x_f32 else f16
    nc = bacc.Bacc("TRN2", target_bir_lowering=False, debug=False,
                   num_devices=N_CORES)
    x_d = nc.dram_tensor("x", [C, s_loc], xdt, kind="ExternalInput")
    xt_d = nc.dram_tensor("xt", [128, n_sb * W], xdt, kind="ExternalInput")
    qT_d = nc.dram_tensor("qT", [C, HQ], qdt, kind="ExternalInput")
    y_d = nc.dram_tensor("y", [128, 2 * HQ], f32, kind="ExternalOutput")
    l_d = nc.dram_tensor("l", [1, 512], f32, kind="ExternalOutput")

    with tile.TileContext(nc) as tc:
        with (
            tc.tile_pool(name="const", bufs=1) as constp,
            tc.tile_pool(name="xstage", bufs=3) as xstage,
            tc.tile_pool(name="xtstage", bufs=3) as xtstage,
            tc.tile_pool(name="ptstage", bufs=3) as ptstage,
            tc.tile_pool(name="outp", bufs=2) as outp,
            tc.tile_pool(name="ps_st", bufs=3, space="PSUM") as ps_st,
            tc.tile_pool(name="ps_y", bufs=2, space="PSUM") as ps_yp,
            tc.tile_pool(name="ps_l", bufs=2, space="PSUM") as ps_lp,
        ):
            qt0 = constp.tile([128, HQ], qdt)
            nc.sync.dma_start(qt0[:], qT_d[0:128, :])
            qt1 = constp.tile([128, HQ], qdt)
            nc.sync.dma_start(qt1[:], qT_d[128:256, :])
            ones = constp.tile([128, 1], f16)
            nc.gpsimd.memset(ones[:], 1.0)

            q0 = nc.sync
            q1 = nc.scalar if multi_queue else nc.sync

            def one_pass():
                psum_y = ps_yp.tile([128, 2 * HQ], f32, tag="psy")
                psum_l = ps_lp.tile([1, 512], f32, tag="psl")

                def emit_l(kk, pt_c):
                    nc.tensor.matmul(
                        psum_l[:], ones[:], pt_c[:],
                        start=(kk == 0), stop=(kk == n_ch - 1),
                        skip_group_check=True)

                def emit_mm2(kk, pt_c, xt_t, xt_lo):
                    for sb in range(sbpc):
                        gsb = kk * sbpc + sb
                        for h in (0, 1):
                            nc.tensor.matmul(
                                psum_y[:, h * HQ:(h + 1) * HQ],
                                xt_t[:, xt_lo + sb * W + h * 128:
                                     xt_lo + sb * W + (h + 1) * 128],
                                pt_c[:, sb * HQ:(sb + 1) * HQ],
                                start=(gsb == 0 and h == 0),
                                stop=(gsb == n_sb - 1 and h == 1),
                                skip_group_check=True)

                pending = None  # (k, pt_c, xt_t, xt_lo) awaiting l+mm2
                xc0_t = xc1_t = xt_t = None
                for k in range(n_ch):
                    if k % per_dma == 0:
                        d_o = k * chunk
                        xc0_t = xstage.tile([128, dma_chunk], xdt, tag="xc0")
                        q0.dma_start(xc0_t[:], x_d[0:128, d_o:d_o + dma_chunk])
                        xc1_t = xstage.tile([128, dma_chunk], xdt, tag="xc1")
                        q0.dma_start(xc1_t[:], x_d[128:256, d_o:d_o + dma_chunk])
                        dcols = (dma_chunk // 128) * W
                        xt_t = xtstage.tile([128, dcols], xdt, tag="xt")
                        ocol = (d_o // 128) * W
                        q1.dma_start(xt_t[:], xt_d[:, ocol:ocol + dcols])
                    lo = (k % per_dma) * chunk
                    xt_lo = (k % per_dma) * sbpc * W

                    # scoresT: 16 s-blocks packed into one [128, 512] bank
                    pst = ps_st.tile([128, sbpc * HQ], f32, tag="pst")
                    for sb in range(sbpc):
                        for h, (xc, qt) in enumerate(((xc0_t, qt0), (xc1_t, qt1))):
                            nc.tensor.matmul(
                                pst[:, sb * HQ:(sb + 1) * HQ],
                                xc[:, lo + sb * 128:lo + (sb + 1) * 128], qt[:],
                                start=(sb == 0 and h == 0),
                                stop=(sb == sbpc - 1 and h == 1),
                                skip_group_check=True)
                    pt_c = ptstage.tile([128, sbpc * HQ], f16, tag="pt")
                    nc.scalar.activation(pt_c[:], pst[:], Exp, scale=SCALE)
                    if pipe:
                        # l+mm2 run one sub-chunk behind so exp(k) overlaps
                        # mm1(k+1) instead of stalling PE.
                        if pending is not None:
                            emit_l(*pending[:2])
                            emit_mm2(*pending)
                        pending = (k, pt_c, xt_t, xt_lo)
                    else:
                        emit_l(k, pt_c)
                        emit_mm2(k, pt_c, xt_t, xt_lo)
                if pending is not None:
                    emit_l(*pending[:2])
                    emit_mm2(*pending)

                y_t = outp.tile([128, 2 * HQ], f32, tag="yt")
                nc.vector.tensor_copy(y_t[:], psum_y[:])
                l_t = outp.tile([1, 512], f32, tag="lt")
                nc.vector.tensor_copy(l_t[:], psum_l[:])
                nc.sync.dma_start(y_d[:], y_t[:])
                nc.sync.dma_start(l_d[:], l_t[:])

            if loop_n > 1:
                E = mybir.EngineType
                hints = (E.PE, E.DVE, E.Activation, E.SP, E.Pool)

                def unrollable_body(iv0, u):
                    for _ in range(u):
                        one_pass()

                # For_i places an all-engine barrier on every back-edge,
                # draining the DMA/compute pipeline each pass; unrolling
                # amortizes it and lets consecutive passes overlap.
                tc.For_i_unrolled_general(
                    start=0, end=loop_n, step=1,
                    unrollable_body=unrollable_body,
                    max_unroll=unroll, hint_engines=hints)
            else:
                one_pass()

    nc.compile()
    return nc


def _get_nc(loop_n=1, x_f32=False, chunk=CHUNK, dma_chunk=4096,
            multi_queue=True, pipe=True, unroll=4):
    key = (S_LOC, loop_n, x_f32, chunk, dma_chunk, multi_queue, pipe, unroll)
    if key not in _NC_CACHE:
        _NC_CACHE[key] = _build_nc(loop_n=loop_n, x_f32=x_f32, chunk=chunk,
                                   dma_chunk=dma_chunk,
                                   multi_queue=multi_queue, pipe=pipe,
                                   unroll=unroll)
    return _NC_CACHE[key]


def _shard_inputs(shard, qT, s_loc=S_LOC, chunk=CHUNK, x_f32=False):
    """shard: [C, s_loc] fp32 -> in_map for one core."""
    import ml_dtypes

    n_sb = s_loc // 128
    xdt = np.float32 if x_f32 else ml_dtypes.float8_e3m4
    x8 = shard.astype(xdt)
    # flat transposed plane: column block j (width C) = shard[:, j*128:+128].T
    xt = np.ascontiguousarray(
        shard.T.reshape(n_sb, 128, C).astype(xdt)
        .transpose(1, 0, 2).reshape(128, n_sb * C))
    return {"x": np.ascontiguousarray(x8), "xt": xt,
            "qT": qT.astype(np.float32 if x_f32 else np.float16)}


def _prepare_in_maps(x, queries, Wk, x_f32=False):
    xf = np.ascontiguousarray(np.asarray(x, np.float32).reshape(B, C, S))
    qr = np.asarray(queries, np.float32).reshape(NUM_QUERIES, NUM_HEADS, HEAD_DIM)
    Wkr = np.asarray(Wk, np.float32).reshape(NUM_HEADS, HEAD_DIM, C)
    # q_eff[h*NQ+q, c] = sum_d q[q,h,d] * Wk[h*hd+d, c]
    q_eff = np.einsum("qhd,hdc->hqc", qr, Wkr).reshape(HQ, C)
    qT = np.ascontiguousarray(q_eff.T.astype(np.float32))
    in_maps = []
    for core in range(N_CORES):
        b, half = divmod(core, 2)
        shard = np.ascontiguousarray(xf[b, :, half * S_LOC:(half + 1) * S_LOC])
        in_maps.append(_shard_inputs(shard, qT, x_f32=x_f32))
    return in_maps


def _extract_yl(yv, lv):
    """Device outputs -> (Y [HQ, C], L [HQ]) for one core."""
    Y = np.concatenate([yv[:, 0:HQ].T, yv[:, HQ:2 * HQ].T], axis=1)  # [HQ, 256]
    L = lv.reshape(-1, HQ).sum(axis=0)
    return Y, L


def _epilogue(Y, L, Wv, bv, Wo, bo, gamma, beta):
    """Y [B, HQ, C], L [B, HQ] -> final [B, OUT_FEATURES]."""
    pooled = (Y / L[:, :, None]).reshape(B, NUM_HEADS, NUM_QUERIES, C)
    Wvr = np.asarray(Wv, np.float32).reshape(NUM_HEADS, HEAD_DIM, C)
    att = np.einsum("hdc,bhqc->bhqd", Wvr, pooled)
    att += np.asarray(bv, np.float32).reshape(1, NUM_HEADS, 1, HEAD_DIM)
    multi = att.transpose(0, 2, 1, 3).reshape(B, NUM_QUERIES * OUT_FEATURES)
    out = multi @ np.asarray(Wo, np.float32).T + np.asarray(bo, np.float32)
    mu = out.mean(-1, keepdims=True)
    var = ((out - mu) ** 2).mean(-1, keepdims=True)
    out = (out - mu) / np.sqrt(var + LN_EPS)
    out = out * np.asarray(gamma, np.float32) + np.asarray(beta, np.float32)
    return out.astype(np.float32)


def kernel(x, queries, Wk, bk, Wv, bv, Wo, bo, gamma, beta):
    from concourse.bass_utils import run_bass_kernel_spmd

    in_maps = _prepare_in_maps(x, queries, Wk)
    nc = _get_nc()
    res = run_bass_kernel_spmd(nc, in_maps, list(range(N_CORES))).results
    Y = np.zeros((B, HQ, C), np.float32)
    L = np.zeros((B, HQ), np.float32)
    for core in range(N_CORES):
        b = core // 2
        Yc, Lc = _extract_yl(res[core]["y"], res[core]["l"])
        Y[b] += Yc
        L[b] += Lc
    return _epilogue(Y, L, Wv, bv, Wo, bo, gamma, beta)



# revision 15
# speedup vs baseline: 1.3206x; 1.1172x over previous
"""MultiHeadAttentionPool3D on 8 Trainium2 NeuronCores.

Math (per batch b):
  scores[hq, s] = scale * (q_eff[hq, :] @ x[b, :, s])     (key-projection folded into
                                                           the queries; per-row bias
                                                           terms cancel in softmax)
  p = exp(scores)       (no max-subtraction: scores ~ N(0,1), fp32-safe)
  l[hq] = sum_s p[hq, s];   y[hq, c] = sum_s p[hq, s] * x[b, c, s]
  pooled = y / l  -> tiny epilogue (value proj, Wo, layernorm) on host.

Sharding: core = b * 2 + s_half  (4 batches x 2 halves of S=32768).

v9 design (both matmuls in small-output orientation):
  - host passes TWO fp16 layouts of the shard: x [C, S_loc] (c on partitions)
    and a flat transposed plane xt [128, n_sb*256] (s on partitions; column
    block j = x[:, j*128:(j+1)*128].T).
  - scoresT per s-block DIRECTLY via x-tile-as-stationary:
      psum_sT[:, sb*32:+32] += x_half[:, sb*128:+128].T @ q_effT_half
    16 s-blocks packed per [128, 512] PSUM bank; the output IS pT-oriented,
    so p never needs transposing.
  - p = Exp(scale*scoresT): one [128,512] ScalarE activation per chunk -> fp16.
  - l partials: ones[128,1].T @ pT_chunk -> psum_l [1, 512], accumulated
    across chunks (host sums the 16 slots).
  - y: per s-block and c-half: xt_slice[s,128].T @ pT_slice[s,32]
    -> psum_y [128(c-half), 64(2*hq)], PSUM-accumulated over all s.
  - host reassembles y/l and runs the tiny epilogue.
"""

import sys

if "/opt/trn_rl_repo" not in sys.path:
    sys.path.insert(0, "/opt/trn_rl_repo")

import numpy as np

NUM_HEADS = 8
OUT_FEATURES = 512
NUM_QUERIES = 4
C = 256
HEAD_DIM = OUT_FEATURES // NUM_HEADS
LN_EPS = 1e-5
B = 4
S = 32 * 32 * 32
N_CORES = 8
S_LOC = S // 2  # shard: (batch, half of spatial axis)
HQ = NUM_HEADS * NUM_QUERIES  # 32 fused query rows, hq = h*NUM_QUERIES + q
SCALE = HEAD_DIM ** -0.5
CHUNK = 2048

_NC_CACHE = {}


def _build_nc(s_loc=S_LOC, chunk=CHUNK, dma_chunk=4096, loop_n=1, x_f32=False,
              multi_queue=True, pipe=True, unroll=8, xt_prefetch=True):
    import concourse.bass as bass
    import concourse.tile as tile
    from concourse import bacc, mybir
    import contextlib

    f32 = mybir.dt.float32
    f16 = mybir.dt.float16
    # x planes travel as fp8 e3m4 (halves HBM traffic); q_eff and p stay
    # fp16 — mixed-dtype matmuls (fp8 stationary x fp16 moving) are exact
    # on HW and q/p quantization is the accuracy-critical path.
    xdt = f32 if x_f32 else mybir.dt.float8e3
    Exp = mybir.ActivationFunctionType.Exp

    assert s_loc % dma_chunk == 0 and dma_chunk % chunk == 0
    assert chunk == 2048, "pst/l packing assumes 16 s-blocks per sub-chunk"
    n_ch = s_loc // chunk
    per_dma = dma_chunk // chunk
    n_sb = s_loc // 128
    W = 256
    sbpc = chunk // 128

    qdt = f32 if x_f32 else f16
    nc = bacc.Bacc("TRN2", target_bir_lowering=False, debug=False,
                   num_devices=N_CORES)
    x_d = nc.dram_tensor("x", [C, s_loc], xdt, kind="ExternalInput")
    xt_d = nc.dram_tensor("xt", [128, n_sb * W], xdt, kind="ExternalInput")
    qT_d = nc.dram_tensor("qT", [C, HQ], qdt, kind="ExternalInput")
    y_d = nc.dram_tensor("y", [128, 2 * HQ], f32, kind="ExternalOutput")
    l_d = nc.dram_tensor("l", [1, 512], f32, kind="ExternalOutput")

    with tile.TileContext(nc) as tc:
        with (
            tc.tile_pool(name="const", bufs=1) as constp,
            tc.tile_pool(name="xstage", bufs=3) as xstage,
            tc.tile_pool(name="xtstage", bufs=5 if xt_prefetch else 3) as xtstage,
            tc.tile_pool(name="ptstage", bufs=3) as ptstage,
            tc.tile_pool(name="outp", bufs=2) as outp,
            tc.tile_pool(name="ps_st", bufs=3, space="PSUM") as ps_st,
            tc.tile_pool(name="ps_y", bufs=2, space="PSUM") as ps_yp,
            tc.tile_pool(name="ps_l", bufs=2, space="PSUM") as ps_lp,
        ):
            qt0 = constp.tile([128, HQ], qdt)
            nc.sync.dma_start(qt0[:], qT_d[0:128, :])
            qt1 = constp.tile([128, HQ], qdt)
            nc.sync.dma_start(qt1[:], qT_d[128:256, :])
            ones = constp.tile([128, 1], f16)
            nc.gpsimd.memset(ones[:], 1.0)

            q0 = nc.sync
            q1 = nc.scalar if multi_queue else nc.sync

            def one_pass():
                psum_y = ps_yp.tile([128, 2 * HQ], f32, tag="psy")
                psum_l = ps_lp.tile([1, 512], f32, tag="psl")

                def emit_l(kk, pt_c):
                    nc.tensor.matmul(
                        psum_l[:], ones[:], pt_c[:],
                        start=(kk == 0), stop=(kk == n_ch - 1),
                        skip_group_check=True)

                def emit_mm2(kk, pt_c, xt_t, xt_lo):
                    for sb in range(sbpc):
                        gsb = kk * sbpc + sb
                        for h in (0, 1):
                            nc.tensor.matmul(
                                psum_y[:, h * HQ:(h + 1) * HQ],
                                xt_t[:, xt_lo + sb * W + h * 128:
                                     xt_lo + sb * W + (h + 1) * 128],
                                pt_c[:, sb * HQ:(sb + 1) * HQ],
                                start=(gsb == 0 and h == 0),
                                stop=(gsb == n_sb - 1 and h == 1),
                                skip_group_check=True)

                dcols = (dma_chunk // 128) * W
                xts = []
                if xt_prefetch:
                    # all xt transfers dispatched up front: ACT's HWDGE
                    # dispatches never interleave with its exp work
                    for d in range(n_ch // per_dma):
                        xt_t = xtstage.tile([128, dcols], xdt, tag="xt")
                        q1.dma_start(xt_t[:], xt_d[:, d * dcols:(d + 1) * dcols])
                        xts.append(xt_t)

                pending = None  # (k, pt_c, xt_t, xt_lo) awaiting l+mm2
                xc0_t = xc1_t = xt_t = None
                for k in range(n_ch):
                    if k % per_dma == 0:
                        d_o = k * chunk
                        xc0_t = xstage.tile([128, dma_chunk], xdt, tag="xc0")
                        q0.dma_start(xc0_t[:], x_d[0:128, d_o:d_o + dma_chunk])
                        xc1_t = xstage.tile([128, dma_chunk], xdt, tag="xc1")
                        q0.dma_start(xc1_t[:], x_d[128:256, d_o:d_o + dma_chunk])
                        if xt_prefetch:
                            xt_t = xts[k // per_dma]
                        else:
                            xt_t = xtstage.tile([128, dcols], xdt, tag="xt")
                            ocol = (d_o // 128) * W
                            q1.dma_start(xt_t[:], xt_d[:, ocol:ocol + dcols])
                    lo = (k % per_dma) * chunk
                    xt_lo = (k % per_dma) * sbpc * W

                    # scoresT: 16 s-blocks packed into one [128, 512] bank
                    pst = ps_st.tile([128, sbpc * HQ], f32, tag="pst")
                    for sb in range(sbpc):
                        for h, (xc, qt) in enumerate(((xc0_t, qt0), (xc1_t, qt1))):
                            nc.tensor.matmul(
                                pst[:, sb * HQ:(sb + 1) * HQ],
                                xc[:, lo + sb * 128:lo + (sb + 1) * 128], qt[:],
                                start=(sb == 0 and h == 0),
                                stop=(sb == sbpc - 1 and h == 1),
                                skip_group_check=True)
                    pt_c = ptstage.tile([128, sbpc * HQ], f16, tag="pt")
                    nc.scalar.activation(pt_c[:], pst[:], Exp, scale=SCALE)
                    if pipe:
                        # l+mm2 run one sub-chunk behind so exp(k) overlaps
                        # mm1(k+1) instead of stalling PE.
                        if pending is not None:
                            emit_l(*pending[:2])
                            emit_mm2(*pending)
                        pending = (k, pt_c, xt_t, xt_lo)
                    else:
                        emit_l(k, pt_c)
                        emit_mm2(k, pt_c, xt_t, xt_lo)
                if pending is not None:
                    emit_l(*pending[:2])
                    emit_mm2(*pending)

                y_t = outp.tile([128, 2 * HQ], f32, tag="yt")
                nc.vector.tensor_copy(y_t[:], psum_y[:])
                l_t = outp.tile([1, 512], f32, tag="lt")
                nc.vector.tensor_copy(l_t[:], psum_l[:])
                nc.sync.dma_start(y_d[:], y_t[:])
                nc.sync.dma_start(l_d[:], l_t[:])

            if loop_n > 1:
                E = mybir.EngineType
                hints = (E.PE, E.DVE, E.Activation, E.SP, E.Pool)

                def unrollable_body(iv0, u):
                    for _ in range(u):
                        one_pass()

                # For_i places an all-engine barrier on every back-edge,
                # draining the DMA/compute pipeline each pass; unrolling
                # amortizes it and lets consecutive passes overlap.
                tc.For_i_unrolled_general(
                    start=0, end=loop_n, step=1,
                    unrollable_body=unrollable_body,
                    max_unroll=unroll, hint_engines=hints)
            else:
                one_pass()

    nc.compile()
    return nc


def _get_nc(loop_n=1, x_f32=False, chunk=CHUNK, dma_chunk=4096,
            multi_queue=True, pipe=True, unroll=8, xt_prefetch=True):
    key = (S_LOC, loop_n, x_f32, chunk, dma_chunk, multi_queue, pipe, unroll,
           xt_prefetch)
    if key not in _NC_CACHE:
        _NC_CACHE[key] = _build_nc(loop_n=loop_n, x_f32=x_f32, chunk=chunk,
                                   dma_chunk=dma_chunk,
                                   multi_queue=multi_queue, pipe=pipe,
                                   unroll=unroll, xt_prefetch=xt_prefetch)
    return _NC_CACHE[key]


def _shard_inputs(shard, qT, s_loc=S_LOC, chunk=CHUNK, x_f32=False):
    """shard: [C, s_loc] fp32 -> in_map for one core."""
    import ml_dtypes

    n_sb = s_loc // 128
    xdt = np.float32 if x_f32 else ml_dtypes.float8_e3m4
    x8 = shard.astype(xdt)
    # flat transposed plane: column block j (width C) = shard[:, j*128:+128].T
    xt = np.ascontiguousarray(
        shard.T.reshape(n_sb, 128, C).astype(xdt)
        .transpose(1, 0, 2).reshape(128, n_sb * C))
    return {"x": np.ascontiguousarray(x8), "xt": xt,
            "qT": qT.astype(np.float32 if x_f32 else np.float16)}


def _prepare_in_maps(x, queries, Wk, x_f32=False):
    xf = np.ascontiguousarray(np.asarray(x, np.float32).reshape(B, C, S))
    qr = np.asarray(queries, np.float32).reshape(NUM_QUERIES, NUM_HEADS, HEAD_DIM)
    Wkr = np.asarray(Wk, np.float32).reshape(NUM_HEADS, HEAD_DIM, C)
    # q_eff[h*NQ+q, c] = sum_d q[q,h,d] * Wk[h*hd+d, c]
    q_eff = np.einsum("qhd,hdc->hqc", qr, Wkr).reshape(HQ, C)
    qT = np.ascontiguousarray(q_eff.T.astype(np.float32))
    in_maps = []
    for core in range(N_CORES):
        b, half = divmod(core, 2)
        shard = np.ascontiguousarray(xf[b, :, half * S_LOC:(half + 1) * S_LOC])
        in_maps.append(_shard_inputs(shard, qT, x_f32=x_f32))
    return in_maps


def _extract_yl(yv, lv):
    """Device outputs -> (Y [HQ, C], L [HQ]) for one core."""
    Y = np.concatenate([yv[:, 0:HQ].T, yv[:, HQ:2 * HQ].T], axis=1)  # [HQ, 256]
    L = lv.reshape(-1, HQ).sum(axis=0)
    return Y, L


def _epilogue(Y, L, Wv, bv, Wo, bo, gamma, beta):
    """Y [B, HQ, C], L [B, HQ] -> final [B, OUT_FEATURES]."""
    pooled = (Y / L[:, :, None]).reshape(B, NUM_HEADS, NUM_QUERIES, C)
    Wvr = np.asarray(Wv, np.float32).reshape(NUM_HEADS, HEAD_DIM, C)
    att = np.einsum("hdc,bhqc->bhqd", Wvr, pooled)
    att += np.asarray(bv, np.float32).reshape(1, NUM_HEADS, 1, HEAD_DIM)
    multi = att.transpose(0, 2, 1, 3).reshape(B, NUM_QUERIES * OUT_FEATURES)
    out = multi @ np.asarray(Wo, np.float32).T + np.asarray(bo, np.float32)
    mu = out.mean(-1, keepdims=True)
    var = ((out - mu) ** 2).mean(-1, keepdims=True)
    out = (out - mu) / np.sqrt(var + LN_EPS)
    out = out * np.asarray(gamma, np.float32) + np.asarray(beta, np.float32)
    return out.astype(np.float32)


def kernel(x, queries, Wk, bk, Wv, bv, Wo, bo, gamma, beta):
    from concourse.bass_utils import run_bass_kernel_spmd

    in_maps = _prepare_in_maps(x, queries, Wk)
    nc = _get_nc()
    res = run_bass_kernel_spmd(nc, in_maps, list(range(N_CORES))).results
    Y = np.zeros((B, HQ, C), np.float32)
    L = np.zeros((B, HQ), np.float32)
    for core in range(N_CORES):
        b = core // 2
        Yc, Lc = _extract_yl(res[core]["y"], res[core]["l"])
        Y[b] += Yc
        L[b] += Lc
    return _epilogue(Y, L, Wv, bv, Wo, bo, gamma, beta)

